# revision 49
# baseline (speedup 1.0000x reference)
# Trainium2 Bass kernel for nn_MultiHeadAttention (B=2, L=2048, HID=2048, 16 heads).
#
# Sharding: batch x head-group — core = b*4 + hg handles batch b and heads
# 4*hg..4*hg+3. PE work per core is identical to pure head sharding (4
# (batch,head) pairs each), but x loads halve (one batch per core) and the
# out-projection partial is [L, HID] per batch instead of [B, L, HID]; the
# host sums the 4 partials per batch. Partials are stored bf16 (the rel-err
# budget is 2e-2; bf16 partial rounding adds ~4e-3).
#
# Per-core layout choices:
#   - x is shipped pre-transposed for its batch (xT = x[b].T, [HID, L], bf16).
#   - Q^T / K^T are computed directly in [D=128, L] layout (head dim on
#     partitions) so score tiles S^T[k,q] come out of the PE ready to be
#     used as PV-matmul stationary operands — no transposes in the softmax
#     path.
#   - RoPE: the head dim of Wq/Wk is permuted even-first on the host, which
#     turns the reference's interleaved rotation into a half-swap + two
#     elementwise multiply-adds with precomputed [128, L] cos/sin tables.
#   - RMSNorm: folded into a per-position scale. The q-side scale (which
#     also absorbs 1/sqrt(D)) is applied to Q^T before RoPE; the k-side
#     scale is applied as the per-partition `scale` operand of the exp
#     activation on S^T tiles.
#   - Softmax denominator: a ones-column appended to V (V_aug[:,128] = 1)
#     so the PV matmul accumulates the denominator for free.
#   - Causal mask: S^T tiles strictly above the diagonal are skipped
#     entirely; diagonal-straddling tiles get a [128,128] additive mask on
#     the diagonal block. The dead prefix columns of E are never read by
#     the PV matmuls (c <= s implies si >= r), so no zero-fill is needed.
#   - Transposes (V^T -> V, attn -> attn^T) run on the PE via identity
#     matmuls (DMA transposes serialize against copy-mode DMAs).
#   - dtypes: bf16 for the big projections, scores, PV and all RoPE
#     intermediates (full PE rate, 2x DVE rate).
#   - Loads: x per-chunk on the sync HWDGE ring in parallel with the
#     weights per-chunk on SWDGE, so the load-bound startup uses two DMA
#     paths and QKV chains start as chunks arrive.
#   - The output projection is interleaved into the last head's attention
#     J-loop (q-rows 512J.. are final there), so its matmuls fill PE gaps
#     and the SWDGE stores overlap compute instead of forming a tail.
#   - Output partials are stored as 1 MB SWDGE (nc.gpsimd) DMAs with 8 KB
#     per-partition descriptor lines (staging layout [8, 128, 4096] bf16;
#     the host de-interleaves). Measured on this setup: HWDGE stores are
#     ~40 GB/s flat and 4 KB-line stores collapse on any ring, while
#     big-line SWDGE stores reach several hundred GB/s.

import numpy as np
import ml_dtypes

B, L, HID, NH, D = 2, 2048, 2048, 16, 128
NCORES = 8
NQG = 1                     # L-blocks accumulated concurrently in QKV proj
HPC = 4                     # heads per core (1 batch x 4 heads)
NCH = HID // 128            # 16 contraction chunks
NL = 512                    # L tile (free dim) for projections / S tiles
NLB = L // NL               # 4 L-blocks
ROPE_BASE = 10000.0
EPS = 1e-5
MASK_VAL = -1e9

_BF16 = ml_dtypes.bfloat16
_cache = {}


def _host_constants():
    if "consts" in _cache:
        return _cache["consts"]
    # RoPE tables in the even-first permuted basis.
    i = np.arange(64, dtype=np.float64)
    inv_freq = ROPE_BASE ** (-2.0 * i / D)                     # [64]
    ang = np.arange(L, dtype=np.float64)[:, None] * inv_freq   # [L, 64]
    cos, sin = np.cos(ang).T, np.sin(ang).T                    # [64, L]
    csa = np.concatenate([cos, cos], axis=0).astype(_BF16)    # [128, L]
    csb = np.concatenate([-sin, sin], axis=0).astype(_BF16)   # [128, L]
    _cache["consts"] = (csa, csb)
    return _cache["consts"]


def _build_nc():
    if "nc" in _cache:
        return _cache["nc"]
    import concourse.bass as bass  # noqa: F401
    from concourse import bacc
    import concourse.tile as tile
    import concourse.mybir as mybir
    from contextlib import ExitStack

    f32 = mybir.dt.float32
    i32 = mybir.dt.int32
    f32r = mybir.dt.float32r
    bf16 = mybir.dt.bfloat16
    EXP = mybir.ActivationFunctionType.Exp

    nc = bacc.Bacc("TRN2", target_bir_lowering=False, debug=False,
                   enable_asserts=True)
    xT = nc.dram_tensor("xT", [HID, L], bf16, kind="ExternalInput").ap()
    wqkvT = nc.dram_tensor("wqkvT", [HID, 3 * HPC * D], bf16,
                           kind="ExternalInput").ap()
    woutT = nc.dram_tensor("woutT", [HPC * D, HID], bf16,
                           kind="ExternalInput").ap()
    csa_d = nc.dram_tensor("csa", [D, L], bf16, kind="ExternalInput").ap()
    csb_d = nc.dram_tensor("csb", [D, L], bf16, kind="ExternalInput").ap()
    # Staging layout: out[sg, p, cq*HID + f] = out_partial[sg*256 + cq*128 + p, f]
    # -> per-partition lines are 4096 bf16 = 8 KB contiguous.
    out_d = nc.dram_tensor("out", [L // 256, 128, 2 * HID], bf16,
                           kind="ExternalOutput").ap()

    WCOLS = 3 * HPC * D         # 1536 qkv columns per contraction chunk

    with tile.TileContext(nc) as tc, ExitStack() as ctx:
        cpool = ctx.enter_context(tc.tile_pool(name="consts", bufs=1))
        atpool = ctx.enter_context(tc.tile_pool(name="at", bufs=1))
        # Pools live only through the head loop; released before the
        # out-projection so its staging can reuse the x/e space.
        hctx = ExitStack()
        xpool = hctx.enter_context(tc.tile_pool(name="x", bufs=1))
        qkpool = hctx.enter_context(tc.tile_pool(name="qk", bufs=1))
        qk1pool = hctx.enter_context(tc.tile_pool(name="qk1", bufs=1))
        epool = hctx.enter_context(tc.tile_pool(name="e", bufs=17))
        anpool = hctx.enter_context(tc.tile_pool(name="an", bufs=4))
        rpool = hctx.enter_context(tc.tile_pool(name="rden", bufs=2))
        # QKV-only staging: released after the last head's projections so
        # the out-projection staging buffer can take its place.
        qstack = ExitStack()
        spool = qstack.enter_context(tc.tile_pool(name="stage", bufs=3))
        s1pool = qstack.enter_context(tc.tile_pool(name="stage1", bufs=1))
        opool = None
        import os
        # "4211" (big=4, score=2, pv=1, small=1): 4 big banks let QKV chains
        # run ahead of the RoPE drain and give the out-projection 4
        # accumulators (each attnT stationary load serves 4 matmuls);
        # model-swept best among 16 configs across two structure revisions.
        pb = os.environ.get("PSUM_CFG", "4211").strip()
        _b, _s, _o, _m = (int(c) for c in pb)
        OPW = max(w for w in (1, 2, 4) if w <= _b)  # out-proj banks | NLB
        ps_big = ctx.enter_context(tc.tile_pool(name="psA", bufs=_b, space="PSUM"))
        ps_s = (ctx.enter_context(tc.tile_pool(name="psSc", bufs=_s, space="PSUM"))
                if _s else None)
        ps_o = ctx.enter_context(tc.tile_pool(name="psO", bufs=_o, space="PSUM"))
        ps_sm = ctx.enter_context(tc.tile_pool(name="psS", bufs=_m, space="PSUM"))
        if ps_s is None:
            ps_s = ps_big

        # ---- constants ----
        # All big loads go through SWDGE (gpsimd): HWDGE on this setup
        # measures ~40 GB/s flat while big-line SWDGE reaches several
        # hundred GB/s. Issue order is chunk-group-major (wt_g then x_g)
        # so the QKV contraction can progress as groups arrive.
        wtg = [cpool.tile([128, 4 * WCOLS], bf16, tag=f"wt{g}", name=f"wt{g}")
               for g in range(4)]
        xg = [xpool.tile([128, 4 * L], bf16, tag=f"xg{g}", name=f"xg{g}")
              for g in range(4)]
        csb = cpool.tile([128, L], bf16, tag="csb")
        csa = cpool.tile([128, L], bf16, tag="csa")
        wo = cpool.tile([128, HPC * HID], bf16, tag="wo")

        # Per-chunk DMAs (0.4-0.5 MB each) so the first QKV chain can start
        # after ~1 MB arrives instead of a whole 3.5 MB chunk group.
        def load_wt(g, eng):
            for c in range(4):
                eng.dma_start(
                    wtg[g][:, c * WCOLS:(c + 1) * WCOLS],
                    wqkvT[(g * 4 + c) * 128:(g * 4 + c + 1) * 128])

        def load_x(g, eng):
            for c in range(4):
                eng.dma_start(
                    xg[g][:, c * L:(c + 1) * L],
                    xT[(g * 4 + c) * 128:(g * 4 + c + 1) * 128])

        # x rides the sync HWDGE ring while the weights ride SWDGE: the two
        # paths run in parallel, roughly halving the load-bound startup
        # (paired A/B on hardware: mixed beat all-SWDGE by ~400 us median).
        for g in range(4):
            load_wt(g, nc.gpsimd)
            load_x(g, nc.sync)
        nc.sync.dma_start(csb[:, :], csb_d[:, :])
        nc.sync.dma_start(csa[:, :], csa_d[:, :])
        nc.gpsimd.dma_start(wo[:, :].rearrange("p (h f) -> p h f", f=HID),
                            woutT.rearrange("(h p) f -> p h f", p=128))
        ident = cpool.tile([128, 128], bf16, tag="ident")
        from concourse.masks import make_identity
        make_identity(nc, ident[:, :])
        # mask128[k, q] = 0 where q >= k else MASK_VAL (strict upper = masked)
        mask128 = cpool.tile([128, 128], f32, tag="mask128")
        nc.gpsimd.memset(mask128[:, :], 0.0)
        nc.gpsimd.affine_select(
            out=mask128[:, :], in_=mask128[:, :],
            compare_op=mybir.AluOpType.is_ge, fill=MASK_VAL,
            base=0, pattern=[[1, 128]], channel_multiplier=-1)
        ones_c32 = cpool.tile([128, 1], f32, tag="ones_c")
        nc.gpsimd.memset(ones_c32[:, :], 1.0)
        ones_cb = cpool.tile([128, 1], bf16, tag="ones_cb")
        nc.gpsimd.memset(ones_cb[:, :], 1.0)

        attnT = [atpool.tile([128, L], bf16, tag=f"attnT{h}",
                             name=f"attnT{h}")
                 for h in range(HPC)]

        for h in range(HPC):
            qr = qkpool.tile([128, L], bf16, tag="qr")
            kr = qkpool.tile([128, L], bf16, tag="kr")
            vT = qk1pool.tile([128, L], bf16, tag="vT")
            va = qk1pool.tile([128, 16 * (D + 1)], bf16, tag="va")
            ckT = qk1pool.tile([128, 16], f32, tag="ckT")

            # ---- q/k/v projections + RMS + RoPE ----
            # Contraction chunk c is the OUTER loop over NQG concurrent
            # PSUM banks: each W chunk is loaded into the PE once per
            # NQG L-blocks (NQG*512 moving rows per Ldweights) and the
            # PE streams the accumulation back-to-back.
            for t, name in ((0, "q"), (1, "k"), (2, "v")):
                wcol = (3 * h + t) * D
                qg = NQG
                for n0 in range(0, NLB, qg):
                    pss = [ps_big.tile([128, NL], f32, tag="big",
                                       name=f"pj{h}{t}_{n0 + i}")
                           for i in range(qg)]
                    for c in range(NCH):
                        for i in range(qg):
                            n = n0 + i
                            nc.tensor.matmul(
                                pss[i][:, :],
                                wtg[c // 4][:, (c % 4) * WCOLS + wcol:
                                            (c % 4) * WCOLS + wcol + D],
                                xg[c // 4][:, (c % 4) * L + n * NL:
                                           (c % 4) * L + (n + 1) * NL],
                                start=(c == 0), stop=(c == NCH - 1))
                    for ii in range(qg):
                        n = n0 + ii
                        ps = pss[ii]
                        if name == "v":
                            nc.scalar.copy(vT[:, n * NL:(n + 1) * NL], ps[:, :])
                            continue
                        # RoPE inputs read the PSUM tile directly: half-swap
                        # via ScalarE (GpSimd cannot read PSUM), csb-multiply
                        # on DVE. sumsq is halfswap-invariant, so compute the
                        # square from sw (SBUF) to spare the PSUM read port.
                        # Intermediates are bf16: 2x DVE rate + half SBUF.
                        sw = spool.tile([128, NL], bf16, tag="sw")
                        nc.scalar.copy(sw[0:64, :], ps[64:128, :])
                        nc.scalar.copy(sw[64:128, :], ps[0:64, :])
                        sq = spool.tile([128, NL], bf16, tag="sq")
                        nc.vector.tensor_mul(sq[:, :], sw[:, :], sw[:, :])
                        m2 = spool.tile([128, NL], bf16, tag="m2")
                        nc.vector.tensor_mul(m2[:, :], ps[:, :],
                                             csb[:, n * NL:(n + 1) * NL])
                        if name == "q":
                            # c_q = 1/sqrt(sumsq + D*eps)  (includes 1/sqrt(D))
                            rrow = ps_sm.tile([1, NL], f32, tag="sm")
                            nc.tensor.matmul(rrow[:, :], ones_cb[:, :], sq[:, :],
                                             start=True, stop=True)
                            srow = s1pool.tile([1, NL], f32, tag="srow")
                            nc.scalar.activation(srow[:, :], rrow[:, :], SQRT,
                                                 bias=eps_q[:, :], scale=1.0)
                            cqrow = s1pool.tile([1, NL], bf16, tag="cqrow")
                            with nc.allow_low_precision(
                                    reason="bf16 q-scale; rel-err budget 2e-2"):
                                nc.vector.reciprocal(cqrow[:, :], srow[:, :])
                            bcs = spool.tile([128, NL], bf16, tag="bcs")
                            nc.gpsimd.partition_broadcast(bcs[:, :], cqrow[:, :])
                        else:
                            # c_k = 1/sqrt(sumsq/D + eps), in [128, 4] per chunk
                            ckp = ps_sm.tile([128, 4], f32, tag="sm")
                            for i in range(4):
                                nc.tensor.matmul(
                                    ckp[:, i:i + 1],
                                    sq[:, i * 128:(i + 1) * 128],
                                    ones_cb[:, :],
                                    start=True, stop=True, skip_group_check=True)
                            cks = s1pool.tile([128, 4], f32, tag="cks")
                            nc.scalar.activation(cks[:, :], ckp[:, :], SQRT,
                                                 bias=eps_k[:, :], scale=1.0 / D)
                            nc.vector.reciprocal(ckT[:, n * 4:(n + 1) * 4],
                                                 cks[:, :])
                        # RoPE: y = csa*halfswap(x) + csb*x  [+ *c_q for q]
                        m1 = spool.tile([128, NL], bf16, tag="m1")
                        nc.vector.tensor_mul(m1[:, :], sw[:, :],
                                             csa[:, n * NL:(n + 1) * NL])
                        dst = qr if name == "q" else kr
                        if name == "q":
                            nc.vector.tensor_add(m1[:, :], m1[:, :], m2[:, :])
                            nc.vector.tensor_mul(dst[:, n * NL:(n + 1) * NL],
                                                 m1[:, :], bcs[:, :])
                        else:
                            nc.vector.tensor_add(dst[:, n * NL:(n + 1) * NL],
                                                 m1[:, :], m2[:, :])

            if h == HPC - 1:
                # QKV staging dead; its SBUF becomes the out-proj staging.
                qstack.close()
                opool = hctx.enter_context(tc.tile_pool(name="ostage", bufs=2))

            # ---- V^T -> V natural with ones column ----
            # PE transposes (DMA transposes serialize against copy-mode
            # DMAs via the xbar-mode hazard, stalling everything).
            nc.gpsimd.memset(va[:, :], 1.0)
            for lc in range(16):
                vtp = ps_sm.tile([128, 128], bf16, tag="sm",
                                 name=f"vtp{h}_{lc}")
                nc.tensor.transpose(vtp[:, :], vT[:, lc * 128:(lc + 1) * 128],
                                    ident[:, :])
                nc.vector.tensor_copy(va[:, lc * 129: lc * 129 + 128],
                                      vtp[:, :])

            # ---- attention ----
            for J in range(NLB):
                etiles = []
                for c in range(4 * J + 4):
                    r = c - 4 * J
                    et = epool.tile([128, NL], bf16, tag="e",
                                    name=f"e{h}{J}_{c}")
                    if r >= 0:
                        # diagonal-straddling tile: columns below
                        # q = 128r are fully masked — skip them in the
                        # matmul; mask the diagonal 128-block; zero-fill
                        # the dead prefix of E.
                        w = NL - r * 128
                        sp = ps_s.tile([128, NL], f32,
                                       tag="s" if ps_s is not ps_big else "big",
                                       name=f"spd{h}{J}_{c}")
                        nc.tensor.matmul(
                            sp[:, 0:w], kr[:, c * 128:(c + 1) * 128],
                            qr[:, J * NL + r * 128:(J + 1) * NL],
                            start=True, stop=True)
                        nc.vector.tensor_add(sp[:, 0:128], sp[:, 0:128],
                                             mask128[:, :])
                        # dead prefix et[:, 0:r*128] is never read: the PV
                        # matmul for s=4J+si only touches tiles c <= s, i.e.
                        # slices si >= r — no zero-fill needed.
                        nc.scalar.activation(et[:, r * 128:], sp[:, 0:w],
                                             EXP, scale=ckT[:, c:c + 1])
                    else:
                        sp = ps_s.tile([128, NL], f32,
                                       tag="s" if ps_s is not ps_big else "big",
                                       name=f"sp{h}{J}_{c}")
                        nc.tensor.matmul(sp[:, :], kr[:, c * 128:(c + 1) * 128],
                                         qr[:, J * NL:(J + 1) * NL],
                                         start=True, stop=True)
                        nc.scalar.activation(et[:, :], sp[:, :],
                                             EXP, scale=ckT[:, c:c + 1])
                    etiles.append(et)
                for si in range(4):
                    s = 4 * J + si
                    op = ps_o.tile([128, D + 1], f32, tag="o")
                    for c in range(s + 1):
                        nc.tensor.matmul(
                            op[:, :],
                            etiles[c][:, si * 128:(si + 1) * 128],
                            va[:, c * 129:(c + 1) * 129],
                            start=(c == 0), stop=(c == s))
                    rden = rpool.tile([128, 1], f32, tag="rden")
                    nc.vector.reciprocal(rden[:, :], op[:, D:D + 1])
                    atn = anpool.tile([128, 128], bf16, tag="atn",
                                      name=f"atn{h}_{s}")
                    nc.vector.tensor_scalar_mul(atn[:, :], op[:, 0:D],
                                                rden[:, :])
                    atp = ps_sm.tile([128, 128], bf16, tag="sm",
                                     name=f"atp{h}_{s}")
                    nc.tensor.transpose(atp[:, :], atn[:, :], ident[:, :])
                    nc.vector.tensor_copy(
                        attnT[h][:, s * 128:(s + 1) * 128], atp[:, :])

                    # ---- output projection (partial over this core's
                    # channels), interleaved with the last head's attention:
                    # q-rows qb*128.. are final once attnT[3] has both their
                    # blocks (after si=1 and si=3), so the projection + SWDGE
                    # store overlap the remaining attention compute instead
                    # of forming a serial tail. Stores are 1 MB SWDGE
                    # (gpsimd) DMAs with 8 KB per-partition lines: HWDGE
                    # stores measure ~40 GB/s flat while big-line SWDGE
                    # stores are ~5-10x that.
                    if h == HPC - 1 and si % 2 == 1:
                        sg = 2 * J + si // 2
                        ob = opool.tile([128, 2 * HID], bf16, tag="ob")
                        for cq in range(2):
                            qb = sg * 2 + cq
                            # f-pairs inside hh across OPW PSUM banks: each
                            # attnT stationary load serves OPW matmuls.
                            for f0 in range(0, NLB, OPW):
                                opjs = [ps_big.tile([128, NL], f32, tag="big",
                                                    name=f"opj{sg}{cq}_{f0+i}")
                                        for i in range(OPW)]
                                for hh in range(HPC):
                                    for i in range(OPW):
                                        f = f0 + i
                                        nc.tensor.matmul(
                                            opjs[i][:, :],
                                            attnT[hh][:, qb * 128:(qb + 1) * 128],
                                            wo[:, hh * HID + f * NL:
                                               hh * HID + (f + 1) * NL],
                                            start=(hh == 0),
                                            stop=(hh == HPC - 1))
                                for i in range(OPW):
                                    f = f0 + i
                                    nc.vector.tensor_copy(
                                        ob[:, (cq * 4 + f) * NL:
                                           (cq * 4 + f + 1) * NL],
                                        opjs[i][:, :])
                        if sg == L // 256 - 1:
                            # split the final store so its descgen+transfer
                            # pipelines with the last copies (shrinks the
                            # end-of-kernel drain)
                            nc.gpsimd.dma_start(out_d[sg][:, 0:HID],
                                                ob[:, 0:HID])
                            nc.gpsimd.dma_start(out_d[sg][:, HID:2 * HID],
                                                ob[:, HID:2 * HID])
                        else:
                            nc.gpsimd.dma_start(out_d[sg], ob[:, :])

        hctx.close()

    nc.compile()
    _cache["nc"] = nc
    return nc


def _prep_in_maps(x, W_qkv, W_out):
    csa, csb = _host_constants()
    perm = np.concatenate([np.arange(0, D, 2), np.arange(1, D, 2)])
    xTb = [np.ascontiguousarray(x[b].T).astype(_BF16) for b in range(B)]
    in_maps = []
    for core in range(NCORES):
        b, hg = core // 4, core % 4
        h0 = HPC * hg
        blocks = []
        for h in range(h0, h0 + HPC):
            wq = W_qkv[h * D:(h + 1) * D, :][perm]
            wk = W_qkv[HID + h * D: HID + (h + 1) * D, :][perm]
            wv = W_qkv[2 * HID + h * D: 2 * HID + (h + 1) * D, :]
            blocks += [wq, wk, wv]
        wqkvT = np.ascontiguousarray(
            np.concatenate(blocks, axis=0).T).astype(_BF16)
        woutT = np.ascontiguousarray(
            W_out[:, h0 * D:(h0 + HPC) * D].T).astype(_BF16)
        in_maps.append({
            "xT": xTb[b], "wqkvT": wqkvT, "woutT": woutT,
            "csa": csa, "csb": csb,
        })
    return in_maps


def kernel(x, W_qkv, W_out):
    from concourse.bass_utils import run_bass_kernel_spmd
    nc = _build_nc()
    in_maps = _prep_in_maps(np.asarray(x, dtype=np.float32),
                            np.asarray(W_qkv, dtype=np.float32),
                            np.asarray(W_out, dtype=np.float32))
    res = run_bass_kernel_spmd(nc, in_maps, core_ids=list(range(NCORES)))
    out = np.zeros((B, L, HID), dtype=np.float64)
    for core in range(NCORES):
        b = core // 4
        part = res.results[core]["out"].astype(np.float64)
        # de-interleave staging layout [8, 128, 2*HID]:
        # part[sg, p, cq*HID + f] = out[sg*256 + cq*128 + p, f]
        part = part.reshape(L // 256, 128, 2, HID).transpose(0, 2, 1, 3)
        out[b] += part.reshape(L, HID)
    return out.astype(np.float32)
